# revision 1
# baseline (speedup 1.0000x reference)
"""Trainium2 Bass kernel for nn_AttachmentPredictor.

Computation (per batch row b):
  head = x[b, :-2, :] @ proj_head + x[b,-2,:] @ proj_prep + x[b,-1,:] @ proj_child
  composed = tanh(head)                      # [T-2, P]
  composed = tanh(composed @ hidden_W[0])
  composed = tanh(composed @ hidden_W[1])
  scores = composed @ scorer                 # [T-2]
  out = where(mask, exp(scores), 0); out /= (sum(out) + 1e-7)

Sharding: pure data parallel, batch 64 -> 8 rows per core on 8 cores.

Key algorithmic point: masked-out tokens contribute exactly zero to the
output (their exp(score) is multiplied by 0 and they are excluded from the
softmax sum), so only masked-in tokens are computed.  The host gathers each
row's masked-in tokens into a compact [PADT] layout (PADT = max row count
rounded up to 128, typically ~56% of T), the device runs the dense pipeline
on the compacted tokens, and the host scatters results back.  The compact
mask (1 for i < count, zero-padded to 16 blocks) drives the same masked
softmax tail as the dense kernel, so padding lanes vanish exactly.

Device layout: all activations transposed [P on partitions, tokens free].
x is shipped bf16 and transposed HBM->SBUF by the DMA xbar engine
(dma_start_transpose) - one 3D-AP DMA covers all 8 contraction blocks of a
token span.  All GEMMs run in bf16 (full rate).  Compact rows are laid out
[prep, child, 14 pad, tokens...], so the per-row prep/child bias columns
are simply columns 0/1 of the transposed tile; the bias is accumulated on
the PE and applied through the activation bias port.

Emission is software-pipelined: chunks are processed in pairs so tanh
latency hides behind the sibling chunk's matmuls; the next row's x
transposes and bias are issued a row ahead; the per-row masked-softmax tail
is interleaved into the next row's trailing chunk group.
"""

import sys

import numpy as np

sys.path.insert(0, "/opt/trn_rl_repo")

B = 64
T = 2048
TH = 2046  # head tokens
D = 1024
P = 512
NCORES = 8
R = B // NCORES  # 8 batch rows per core
KD = D // 128  # 8 contraction chunks for layer 1
KP = P // 128  # 4 contraction chunks for layers 2/3/scorer
FRONT = 16  # prep, child, 14 pad tokens at the head of each compact row
J16 = 16  # score blocks in the (zero-padded) tail

_CACHE = {}


def _chunks(PADT):
    """Token-chunk lengths covering PADT.  A chunk is at most 512 (PSUM
    bank) and must START on a 128 boundary (score columns are 128-token
    blocks inside one chunk); the count is kept even so every chunk has a
    pair partner to hide its tanh latency behind."""
    if PADT <= 512:
        return [PADT]
    if PADT <= 1024:
        return [512, PADT - 512]
    if PADT <= 1536:
        return [512, 256, 256, PADT - 1024]
    return [512, 512, 256, PADT - 1280]


def _build(padts):
    import concourse.bass as bass
    import concourse.mybir as mybir
    import concourse.tile as tile
    from concourse import bacc
    from concourse.masks import make_identity
    from concourse.tile_rust import add_dep_helper

    f32 = mybir.dt.float32
    bf16 = mybir.dt.bfloat16
    u8 = mybir.dt.uint8
    AF = mybir.ActivationFunctionType
    ALU = mybir.AluOpType

    # per row-slot geometry: rows are sorted by masked-in count on the
    # host, so each slot pads only to its own max (16-token granularity).
    TCS = [FRONT + p for p in padts]
    CHSS = [_chunks(p) for p in padts]
    OFFS = [
        [FRONT + sum(chs[:c]) for c in range(len(chs))] for chs in CHSS
    ]
    # chunk groups: pairs hide tanh latency; an odd trailing chunk rides solo
    GRPS = [
        [tuple(g for g in (2 * i, 2 * i + 1) if g < len(chs))
         for i in range((len(chs) + 1) // 2)]
        for chs in CHSS
    ]
    TCMAX = max(TCS)

    nc = bacc.Bacc(
        "TRN2", target_bir_lowering=False, debug=False, num_devices=NCORES
    )

    xs = nc.dram_tensor("xs", [R, TCMAX, D], bf16, kind="ExternalInput").ap()
    w1 = nc.dram_tensor("w1", [D, P], bf16, kind="ExternalInput").ap()
    wp = nc.dram_tensor("wp", [D, P], bf16, kind="ExternalInput").ap()
    wc = nc.dram_tensor("wc", [D, P], bf16, kind="ExternalInput").ap()
    h0 = nc.dram_tensor("h0", [P, P], bf16, kind="ExternalInput").ap()
    h1 = nc.dram_tensor("h1", [P, P], bf16, kind="ExternalInput").ap()
    sc = nc.dram_tensor("sc", [P, 1], bf16, kind="ExternalInput").ap()
    mk = nc.dram_tensor("mk", [R, J16 * 128], u8, kind="ExternalInput").ap()
    out = nc.dram_tensor("out", [R, J16 * 128], f32, kind="ExternalOutput").ap()

    with tile.TileContext(nc) as tc:
        with (
            tc.tile_pool(name="mmp_pool", bufs=6, space="PSUM") as mmp_pool,
            tc.tile_pool(name="scp_pool", bufs=1, space="PSUM") as scp_pool,
            tc.tile_pool(name="tlp_pool", bufs=1, space="PSUM") as tlp_pool,
            tc.tile_pool(name="wpool", bufs=1) as wpool,
            tc.tile_pool(name="cpool", bufs=1) as cpool,
            tc.tile_pool(name="xt_pool", bufs=2) as xt_pool,
            tc.tile_pool(name="y_pool", bufs=2 * KP) as y_pool,
            tc.tile_pool(name="tail_pool", bufs=2) as tail_pool,
        ):
            # ---- transposed x, one tile per row: xt[p, k, t] = x[t, k*128+p]
            #
            # InstDmaTransposeAnt is INVISIBLE to the tile dependency tracker
            # (its ISA-lowered access patterns aren't mapped back to tile
            # regions), so every data edge touching these writes is added
            # explicitly with add_dep_helper: readers wait for the covering
            # transposes (RAW), and a row's transposes wait for the last
            # reader of the ring slot they recycle (WAR).  All transposes
            # stay on the SP queue: cross-queue DMA waits lower to the wrong
            # DMA-completion semaphore.
            xts = {}
            xt_wr = {}  # r -> list of (lo, hi, mybir inst) transpose writes
            last_rd = {}  # r -> last emitted matmul reading xts[r]

            def issue_xt_span(r, lo, hi):
                # One DMA transposes [hi-lo, D] -> [128, KD, hi-lo] via a 3D
                # out AP.  Span bounds must be 16-aligned (xbar tile rows)
                # and < TC so the (k, t) out dims can't merge to 2D.
                assert lo % 16 == 0 and hi % 16 == 0 and hi - lo < TCS[r]
                bi = nc.sync.dma_start_transpose(
                    xts[r][:, :, lo:hi], xs[r, lo:hi, :]
                )
                xt_wr.setdefault(r, []).append((lo, hi, bi.ins))
                if r - 2 in last_rd:
                    add_dep_helper(bi.ins, last_rd[r - 2], reason="xt WAR")

            def issue_xt(r):
                xts[r] = xt_pool.tile(
                    [128, KD, TCS[r]], bf16, tag="xtr", name=f"xt{r}"
                )
                mid = (TCS[r] // 2 // 16) * 16
                issue_xt_span(r, 0, mid)
                issue_xt_span(r, mid, TCS[r])

            def dep_on_xt(mm, r, lo, hi):
                for wlo, whi, di in xt_wr[r]:
                    if wlo < hi and lo < whi:
                        add_dep_helper(mm.ins, di, reason="xt RAW")

            # ---- prologue.  Each DMA holds the SP SEQ until the serialized
            # HWDGE accepts it (~1.2us per DMA), so row 0 is transposed in
            # chunk-sized spans ordered so each input lands just before its
            # consumer: w1+span0 for the first layer-1 group (span0 also
            # carries the prep/child bias columns), wp/wc for the bias,
            # h0/h1 for layers 2/3, trailing spans last.
            xts[0] = xt_pool.tile(
                [128, KD, TCS[0]], bf16, tag="xtr", name="xt0"
            )
            w1t = wpool.tile([128, KD, P], bf16)
            wpt = wpool.tile([128, KD, P], bf16)
            wct = wpool.tile([128, KD, P], bf16)
            h0t = wpool.tile([128, KP, P], bf16)
            h1t = wpool.tile([128, KP, P], bf16)
            sct = wpool.tile([128, KP], bf16)
            mka = wpool.tile([J16, R, 128], u8)

            r0_spans = [
                (OFFS[0][c], OFFS[0][c] + CHSS[0][c])
                for c in range(len(CHSS[0]))
            ]
            r0_spans[0] = (0, r0_spans[0][1])  # include the FRONT columns

            nc.sync.dma_start(w1t[:], w1.rearrange("(k p) q -> p k q", p=128))
            issue_xt_span(0, *r0_spans[0])
            if len(r0_spans) > 1:
                issue_xt_span(0, *r0_spans[1])
            nc.sync.dma_start(wpt[:], wp.rearrange("(k p) q -> p k q", p=128))
            nc.sync.dma_start(wct[:], wc.rearrange("(k p) q -> p k q", p=128))
            nc.sync.dma_start(h0t[:], h0.rearrange("(k p) q -> p k q", p=128))
            nc.sync.dma_start(h1t[:], h1.rearrange("(k p) q -> p k q", p=128))
            for s in r0_spans[2:]:
                issue_xt_span(0, *s)
            nc.sync.dma_start(
                sct[:].unsqueeze(-1), sc.rearrange("(k p) s -> p k s", p=128)
            )
            nc.sync.dma_start(mka[:], mk.rearrange("r (j p) -> j r p", p=128))

            ident_f = cpool.tile([128, 128], f32)
            make_identity(nc, ident_f[:])
            ones128x16 = cpool.tile([128, 16], f32)
            nc.vector.memset(ones128x16[:], 1.0)
            rs128 = cpool.tile([128, 1], f32)
            nc.vector.memset(rs128[:], 0.0)
            biasT = cpool.tile([128, KP, R], f32)

            # PE warm-up: the tensor engine only reaches full clock after
            # ~3us of continuous execution; burn the ramp on dummy identity
            # matmuls during the DMA-bound prologue.
            for i in range(14):
                dmy = tlp_pool.tile([128, 128], f32, tag="tl", name=f"wm{i}")
                nc.tensor.matmul(dmy[:], ident_f[:], ident_f[:])

            # ---- helpers -------------------------------------------------
            def emit_bias(r):
                """biasT[:,m,r] = wp.T @ prep + wc.T @ child from compact
                columns 0/1.  Small PSUM tiles on the 'tl' ring so the bias
                never recycles (waits on) the layer-matmul ring."""
                for m in range(KP):
                    bp = tlp_pool.tile([128, 1], f32, tag="tl", name=f"bp{r}{m}")
                    mb = slice(m * 128, (m + 1) * 128)
                    for k in range(KD):
                        mm = nc.tensor.matmul(
                            bp[:],
                            wpt[:, k, mb],
                            xts[r][:, k, 0:1],
                            start=(k == 0),
                            stop=False,
                        )
                        if m == 0 and k == 0:
                            dep_on_xt(mm, r, 0, 2)
                        last_rd[r] = mm.ins
                    for k in range(KD):
                        mm = nc.tensor.matmul(
                            bp[:],
                            wct[:, k, mb],
                            xts[r][:, k, 1:2],
                            start=False,
                            stop=(k == KD - 1),
                        )
                        last_rd[r] = mm.ins
                    nc.vector.tensor_copy(biasT[:, m, r : r + 1], bp[:])

            def emit_l1_mm(r, c, m):
                t0, L = OFFS[r][c], CHSS[r][c]
                ps = mmp_pool.tile([128, L], f32, tag="mm", name="l1ps")
                mb = slice(m * 128, (m + 1) * 128)
                for k in range(KD):
                    mm = nc.tensor.matmul(
                        ps[:],
                        w1t[:, k, mb],
                        xts[r][:, k, t0 : t0 + L],
                        start=(k == 0),
                        stop=(k == KD - 1),
                    )
                    if m == 0 and k == 0:
                        dep_on_xt(mm, r, t0, t0 + L)
                    last_rd[r] = mm.ins
                return ps

            def emit_l1_tanh(r, c, m, ps, ys):
                y = y_pool.tile([128, CHSS[r][c]], bf16, tag="y1", name="y1")
                nc.scalar.activation(
                    y[:], ps[:], AF.Tanh, bias=biasT[:, m, r : r + 1]
                )
                ys[(c, m)] = y

            def emit_l1(r, c, ys):
                for m in range(KP):
                    ps = emit_l1_mm(r, c, m)
                    emit_l1_tanh(r, c, m, ps, ys)

            def emit_mid(wt, yin, r, c, ys, ytag):
                for m in range(KP):
                    L = CHSS[r][c]
                    ps = mmp_pool.tile([128, L], f32, tag="mm", name="lps")
                    mb = slice(m * 128, (m + 1) * 128)
                    for k in range(KP):
                        nc.tensor.matmul(
                            ps[:],
                            wt[:, k, mb],
                            yin[(c, k)][:],
                            start=(k == 0),
                            stop=(k == KP - 1),
                        )
                    y = y_pool.tile([128, L], bf16, tag=ytag, name=ytag)
                    nc.scalar.activation(y[:], ps[:], AF.Tanh)
                    ys[(c, m)] = y

            def emit_score(sc_ps, y3s, r, c):
                L = CHSS[r][c]
                colbase = (OFFS[r][c] - FRONT) // 128
                for jj in range((L + 127) // 128):
                    w = min(128, L - jj * 128)
                    col = colbase + jj
                    jb = slice(jj * 128, jj * 128 + w)
                    for k in range(KP):
                        nc.tensor.matmul(
                            sc_ps[0:w, col : col + 1],
                            y3s[(c, k)][:, jb],
                            sct[:, k : k + 1],
                            start=(k == 0),
                            stop=(k == KP - 1),
                        )

            # ---- per-row masked-softmax tail, emitted in pieces that are
            # interleaved into the NEXT row's trailing chunk group (where
            # they double as latency fillers for the unpaired chunk).
            tails = {}

            def tail_a(r):
                st = tails[r]
                e_pad = tail_pool.tile([128, 128], f32, tag="esb", name="e_pad")
                nc.scalar.activation(e_pad[:, 0:J16], st["sc_ps"][:], AF.Exp)
                st["e_pad"] = e_pad

            def tail_b(r):
                st = tails[r]
                et_ps = tlp_pool.tile([128, 128], f32, tag="tl", name="et_ps")
                nc.tensor.transpose(et_ps[:], st["e_pad"][:], ident_f[:])
                st["et_ps"] = et_ps

            def tail_c1(r):
                st = tails[r]
                mf = tail_pool.tile([16, 128], f32, tag="mf", name="mf")
                nc.vector.tensor_copy(mf[:], mka[:, r, :])
                me = tail_pool.tile([16, 128], f32, tag="me", name="me")
                nc.vector.tensor_tensor(
                    out=me[:], in0=st["et_ps"][0:16, :], in1=mf[:], op=ALU.mult
                )
                rs = tail_pool.tile([16, 1], f32, tag="rs", name="rs")
                nc.vector.reduce_sum(rs[:], me[:], axis=mybir.AxisListType.X)
                nc.vector.tensor_copy(rs128[0:16, :], rs[:])
                st["me"] = me

            def tail_c2(r):
                st = tails[r]
                rb_ps = tlp_pool.tile([16, 1], f32, tag="tl", name="rb_ps")
                nc.tensor.matmul(rb_ps[:], ones128x16[:], rs128[:])
                st["rb_ps"] = rb_ps

            def tail_d(r):
                st = tails[r]
                rb = tail_pool.tile([16, 1], f32, tag="rb", name="rb")
                nc.vector.tensor_scalar_add(rb[:], st["rb_ps"][:], 1e-7)
                rcp = tail_pool.tile([16, 1], f32, tag="rcp", name="rcp")
                nc.vector.reciprocal(rcp[:], rb[:])
                ot = tail_pool.tile([16, 128], f32, tag="ot", name="ot")
                nc.vector.tensor_scalar_mul(ot[:], st["me"][:], rcp[:])
                nc.sync.dma_start(
                    out[r, :].rearrange("(j p) -> j p", p=128), ot[:]
                )
                del tails[r]

            # ---- main loop ----------------------------------------------
            for r in range(R):
                if r + 1 < R:
                    issue_xt(r + 1)
                if r > 0:
                    tail_a(r - 1)
                sc_ps = scp_pool.tile([128, J16], f32, tag="scps", name="sc_ps")
                # zero the whole tile: columns/partitions beyond this row's
                # valid score range are exp'd then masked, and uninitialized
                # PSUM junk there can be huge -> exp gives Inf -> Inf*0 = NaN
                nc.vector.memset(sc_ps[:], 0.0)
                tails[r] = {"sc_ps": sc_ps}
                for gi, grp in enumerate(GRPS[r]):
                    first, last = gi == 0, gi == len(GRPS[r]) - 1
                    y1s, y2s, y3s = {}, {}, {}
                    if first and r == 0:
                        # row 0: run c0's layer-1 matmuls before the bias so
                        # the PE isn't head-of-line blocked on wp/wc; the c0
                        # tanhs (which need the bias) follow.
                        pss = [emit_l1_mm(0, grp[0], m) for m in range(KP)]
                        emit_bias(0)
                        for m in range(KP):
                            emit_l1_tanh(0, grp[0], m, pss[m], y1s)
                        for c in grp[1:]:
                            emit_l1(r, c, y1s)
                    else:
                        for c in grp:
                            emit_l1(r, c, y1s)
                    if last and r + 1 < R:
                        emit_bias(r + 1)
                    for c in grp:
                        emit_mid(h0t, y1s, r, c, y2s, "y2")
                    if last and r > 0:
                        tail_b(r - 1)
                        tail_c1(r - 1)
                    for c in grp:
                        emit_mid(h1t, y2s, r, c, y3s, "y3")
                    if last and r > 0:
                        tail_c2(r - 1)
                    for c in grp:
                        emit_score(sc_ps, y3s, r, c)
                    if last and r > 0:
                        tail_d(r - 1)

            # final row's tail
            tail_a(R - 1)
            tail_b(R - 1)
            tail_c1(R - 1)
            tail_c2(R - 1)
            tail_d(R - 1)
    nc.compile()
    return nc


def _get_nc(padts=None):
    if padts is None:
        padts = _CACHE.get("last_padts", (1152,) * R)
    padts = tuple(padts)
    _CACHE["last_padts"] = padts
    key = ("nc", padts)
    if key not in _CACHE:
        _CACHE[key] = _build(padts)
    return _CACHE[key]


def _prep(inputs):
    """Compact the masked-in tokens per row, sort rows by count so each
    row slot (shared across the 8 SPMD cores) pads only to its own max;
    returns (in_maps, order, gidx, cnt, padts)."""
    import ml_dtypes

    bf = ml_dtypes.bfloat16
    x = np.asarray(inputs["x"], dtype=np.float32)
    mask = np.asarray(inputs["mask"]).astype(bool)
    head_mask = mask[:, :TH]
    gidx = [np.nonzero(head_mask[b])[0] for b in range(B)]
    cnt = np.array([len(g) for g in gidx])
    order = np.argsort(-cnt, kind="stable")  # slot j <- ranks [8j, 8j+8)
    padts = tuple(
        max(16, int(np.ceil(max(int(cnt[order[NCORES * j]]), 1) / 16)) * 16)
        for j in range(R)
    )
    TC = FRONT + max(padts)

    xc = np.zeros((B, TC, D), dtype=bf)
    for b in range(B):
        xc[b, 0] = x[b, T - 2].astype(bf)
        xc[b, 1] = x[b, T - 1].astype(bf)
        xc[b, FRONT : FRONT + cnt[b]] = x[b, gidx[b]].astype(bf)
    mkc = np.zeros((B, J16 * 128), dtype=np.uint8)
    for b in range(B):
        mkc[b, : cnt[b]] = 1

    w1 = np.ascontiguousarray(np.asarray(inputs["proj_head"], dtype=np.float32).astype(bf))
    wpw = np.ascontiguousarray(np.asarray(inputs["proj_prep"], dtype=np.float32).astype(bf))
    wcw = np.ascontiguousarray(np.asarray(inputs["proj_child"], dtype=np.float32).astype(bf))
    hw = np.asarray(inputs["hidden_W"], dtype=np.float32).astype(bf)
    scw = np.ascontiguousarray(np.asarray(inputs["scorer"], dtype=np.float32).astype(bf))

    in_maps = []
    for i in range(NCORES):
        rows = [order[NCORES * j + i] for j in range(R)]
        in_maps.append(
            {
                "xs": np.ascontiguousarray(xc[rows]),
                "w1": w1,
                "wp": wpw,
                "wc": wcw,
                "h0": np.ascontiguousarray(hw[0]),
                "h1": np.ascontiguousarray(hw[1]),
                "sc": scw,
                "mk": np.ascontiguousarray(mkc[rows]),
            }
        )
    return in_maps, order, gidx, cnt, padts


def _run(inputs, **kwargs):
    from concourse.bass_utils import run_bass_kernel_spmd

    in_maps, order, gidx, cnt, padts = _prep(inputs)
    nc = _get_nc(padts)
    res = run_bass_kernel_spmd(
        nc, in_maps, core_ids=list(range(NCORES)), **kwargs
    )
    full = np.zeros((B, TH), dtype=np.float32)
    for i in range(NCORES):
        oc = res.results[i]["out"]
        for j in range(R):
            b = order[NCORES * j + i]
            full[b, gidx[b]] = oc[j, : cnt[b]]
    return full, res


def kernel(**inputs) -> np.ndarray:
    out, _ = _run(inputs)
    return out

